# revision 42
# baseline (speedup 1.0000x reference)
"""Trainium2 Bass kernel for nn_CompressedCausalAttention.

Sharding: 8 cores = 2 batches x 4 head-groups (2 heads each).
Per-core dataflow (chan-major "T" layouts are (channel partition, seq free)):
  host:    xpe = (x+pe)^T in bf16 (per batch), so the device never sees
           x/pe in f32 and does no adds (DMA 8MB -> 2MB per core).
  phase 1: qT,kT chan-major with bias applied on DVE (tensor_scalar_add,
           keeping ACT free for exps); v seq-major [t, j, h, 128] written
           directly by (xpe^T)-as-lhsT matmuls. Cols 0..63 of each head's
           128-wide v slot are ALL ONES: the AV matmul then emits the
           softmax denominator pre-broadcast 64-wide in rows 0..63, free.
  phase 2: flash-style attention over (i=s-window, j=t-chunk) blocks,
           both heads' scores in one 2-bank PSUM tile, ONE merged exp per
           block on ACT (exp is the throughput co-bottleneck with PE),
           strict-causal staircase applied post-exp as a 0/1 triangle
           multiply on GpSimd (window 0) / DVE (windows 1-3), AV
           accumulation per head with the denominator riding along.
  norm:    1/den straight off AV rows 0..63 via reciprocal_approx_fast
           (DVE, psum in / sbuf out, partition offset 0 - the custom op
           mishandles offset inputs), then one DVE mul -> atn (bf16).
  phase 3: partial output projection outpT = Wc_mine^T-slice @ attnT.
Software pipelining: scores run 2 blocks ahead (PSUM sc-tag rotation
depth 2), AV lags 1 block, and window i's normalize+projection pieces
are spread one-per-block over the first 6 blocks of window i+1; the
final window's projections use the freed AV banks with per-head-split
contractions so they start after head0's normalize alone.
PSUM budget (8 banks): sc 2x2 + av0 2x1 + av1 2x1 = 8.
Host: shards inputs, sums the 4 per-batch partials, adds bc_eff
(v-bias folded through the output projection).
"""

import numpy as np
import ml_dtypes

S, B, C, H = 2048, 2, 512, 8
CC = C // H            # 64
HPC = 2                # heads per core
NCORE = 8
SW = 512               # s window (free dim of score tiles)
TCH = 128              # t chunk (partition dim of score tiles)
NW = S // SW           # 4 windows
TEMP = 1.0 / 8.0       # 1/sqrt(CC)
BIGNEG = -30000.0

_CACHE = {}


def _build_bass():
    import concourse.bass as bass
    import concourse.mybir as mybir
    import concourse.tile as tile
    from concourse import bacc

    f32 = mybir.dt.float32
    bf16 = mybir.dt.bfloat16

    nc = bacc.Bacc("TRN2", target_bir_lowering=False)
    xt = nc.declare_dram_parameter("xt", [4, 128, S], bf16, isOutput=False)
    w3t = nc.declare_dram_parameter("w3t", [128, 4, 384], bf16, isOutput=False)
    b3 = nc.declare_dram_parameter("b3", [128, 2], f32, isOutput=False)
    wct = nc.declare_dram_parameter("wct", [128, C], bf16, isOutput=False)
    tri = nc.declare_dram_parameter("tri", [128, 128], bf16, isOutput=False)
    outp = nc.declare_dram_parameter("outp", [C, S], bf16, isOutput=True)

    Ident = mybir.ActivationFunctionType.Identity
    Exp = mybir.ActivationFunctionType.Exp

    with tile.TileContext(nc) as tc:
        with (
            tc.tile_pool(name="singles", bufs=1) as singles,
            tc.tile_pool(name="pbp", bufs=4) as pbp,
            tc.tile_pool(name="atp", bufs=2) as atp,
            tc.tile_pool(name="rbp", bufs=2) as rbp,
            tc.tile_pool(name="osp", bufs=3) as osp,
            tc.tile_pool(name="ps", bufs=2, space="PSUM") as ps,
        ):
            # ---- inputs: w3t (gates first LDWEIGHTS) leads the Sync
            # queue; window-0 xpe chunks fan out across four engine queues
            # so they land simultaneously; the rest stream on Sync ----
            # w3t split per k-chunk and interleaved with the first-needed
            # xpe chunks so the first q matmul's operands land earliest
            w3t_sb = singles.tile([128, 4, 384], bf16, tag="w3t")
            xpe = singles.tile([128, 4, S], bf16, tag="xpe")
            nc.sync.dma_start(out=w3t_sb[:, 0, :], in_=w3t[:, 0, :])
            nc.sync.dma_start(out=xpe[:, 0, 0:SW], in_=xt[0, :, 0:SW])
            nc.scalar.dma_start(out=xpe[:, 1, 0:SW], in_=xt[1, :, 0:SW])
            nc.gpsimd.dma_start(out=xpe[:, 2, 0:SW], in_=xt[2, :, 0:SW])
            for k in range(1, 4):
                nc.sync.dma_start(out=w3t_sb[:, k, :], in_=w3t[:, k, :])
            nc.sync.dma_start(out=xpe[:, 3, 0:SW], in_=xt[3, :, 0:SW])
            b3_sb = singles.tile([128, 2], f32, tag="b3")
            nc.scalar.dma_start(out=b3_sb, in_=b3[:, :])
            tri_sb = singles.tile([128, 128], bf16, tag="tri")
            nc.gpsimd.dma_start(out=tri_sb, in_=tri[:, :])
            wct_sb = singles.tile([128, C], bf16, tag="wct")
            nc.gpsimd.dma_start(out=wct_sb, in_=wct[:, :])
            for w in range(1, NW):
                sl = slice(w * SW, (w + 1) * SW)
                for k in range(4):
                    nc.sync.dma_start(out=xpe[:, k, sl], in_=xt[k, :, sl])

            qT = singles.tile([128, S], bf16, tag="qT")
            kT = singles.tile([128, S], bf16, tag="kT")
            # v seq-major: [t(128), j(16), h(2), 128]; cols CC..127 of each
            # head slot are all ones, so AV rows CC..127 come out as the
            # softmax denominator already broadcast 64-wide (free on PE).
            vsb = singles.tile([128, 16, HPC, 128], bf16, tag="vsb")
            # big strided memset on GpSimd: keeps DVE free for the window-0
            # q/k bias-adds that gate the qkv pipeline
            nc.gpsimd.memset(vsb[:, :, :, 0:CC], 1.0)

            # p-state pre-warm: dependency-free dummy matmuls start the
            # moment the PE preamble ends (~5.5us), so the clock is fully
            # ramped (3us continuous-busy threshold) and the engine is hot
            # when the first real matmul's DMA lands (~10us). Their garbage
            # PSUM output is overwritten by the first start=True real mm.
            warm = singles.tile([128, SW], bf16, tag="warm")
            nc.vector.memset(warm, 0.0)
            for _ in range(8):
                wp = ps.tile([128, SW], f32, tag="sc", name="wp")
                nc.tensor.matmul(wp, lhsT=warm[:, 0:128], rhs=warm,
                                 start=True, stop=True)

            # ---- phase 1: qkv per window (q/k bias on DVE, ACT is
            # reserved for the attention exps) ----
            for w in range(NW):
                sl = slice(w * SW, (w + 1) * SW)
                qkp = ps.tile([128, 2, SW], f32, tag="sc")
                for blk, dst in ((0, qT), (1, kT)):
                    for k in range(4):
                        nc.tensor.matmul(
                            qkp[:, blk, :],
                            lhsT=w3t_sb[:, k, blk * 128:(blk + 1) * 128],
                            rhs=xpe[:, k, sl],
                            start=(k == 0), stop=(k == 3),
                        )
                    nc.vector.tensor_scalar_add(
                        out=dst[:, sl], in0=qkp[:, blk, :],
                        scalar1=b3_sb[:, blk:blk + 1],
                    )
                vp = ps.tile([128, 4, HPC, CC], f32, tag="sc")
                for tc_ in range(4):
                    t0 = (4 * w + tc_) * TCH
                    for k in range(4):
                        nc.tensor.matmul(
                            vp[:, tc_],
                            lhsT=xpe[:, k, t0:t0 + TCH],
                            rhs=w3t_sb[:, k, 256:384],
                            start=(k == 0), stop=(k == 3),
                        )
                nc.vector.tensor_copy(
                    out=vsb[:, 4 * w:4 * w + 4, :, CC:2 * CC], in_=vp,
                )

            # ---- phase 2+3: attention, flat software-pipelined loop ----
            # per-window block order: j=0 (full, starts the AV accumulation),
            # then the short diagonal blocks (their exp->mask->AV latency is
            # hidden among long neighbors), then long off-diagonal blocks so
            # the window ends with deep PE work in flight
            blocks = []
            win_start = {}
            stop_j = {}
            for i in range(NW):
                js = list(range(4 * i + 4))
                win_start[i] = len(blocks)
                stop_j[i] = js[-1]
                blocks += [(i, j) for j in js]
            NB = len(blocks)
            sc_t = {}
            pb_t = {}
            av_t = {}

            def emit_sc(b):
                if b >= NB or b in sc_t:
                    return
                i, j = blocks[b]
                D = max(0, TCH * j - SW * i)
                t = ps.tile([128, HPC, SW], f32, tag="sc", name=f"sc_{b}")
                for h in range(HPC):
                    nc.tensor.matmul(
                        t[:, h, D:SW],
                        lhsT=kT[h * CC:(h + 1) * CC, j * TCH:(j + 1) * TCH],
                        rhs=qT[h * CC:(h + 1) * CC, i * SW + D:(i + 1) * SW],
                        start=True, stop=True,
                    )
                sc_t[b] = t

            def emit_exp(b):
                i, j = blocks[b]
                D = max(0, TCH * j - SW * i)
                pb = pbp.tile([128, HPC, SW], bf16, tag="pb", name=f"pb_{b}")
                nc.scalar.activation(out=pb[:, :, D:SW], in_=sc_t[b][:, :, D:SW],
                                     func=Exp, scale=TEMP)
                if j >= 4 * i:
                    # strict-causal staircase: zero the masked triangle
                    # post-exp. Window 0 runs it on GpSimd (DVE is busy
                    # with qkv biases/v copies); later windows use DVE,
                    # whose shorter launch latency keeps the lag-1 AV fed
                    # through the short diagonal-block cascade.
                    eng = nc.gpsimd if i == 0 else nc.vector
                    for h in range(HPC):
                        eng.tensor_mul(
                            out=pb[:, h, D:D + TCH], in0=pb[:, h, D:D + TCH],
                            in1=tri_sb,
                        )
                pb_t[b] = pb

            def emit_av(b):
                i, j = blocks[b]
                D = max(0, TCH * j - SW * i)
                if j == 0:
                    av_t[i] = [
                        ps.tile([128, SW], f32, tag=f"av{h}", name=f"av{h}_{i}")
                        for h in range(HPC)
                    ]
                for h in range(HPC):
                    nc.tensor.matmul(
                        av_t[i][h][:, D:SW], lhsT=vsb[:, j, h, :],
                        rhs=pb_t[b][:, h, D:SW],
                        start=(j == 0), stop=(j == stop_j[i]),
                    )
                del pb_t[b]

            # normalize + projection for window i, split into 4 pieces that
            # get interleaved into the next window's block stream
            atn_t = {}

            def emit_norm_h(i, h):
                # v cols 0..63 are all ones, so AV rows 0..63 come out as
                # the softmax denominator already broadcast 64-wide
                rcb = rbp.tile([CC, SW], f32, tag="rcb", name=f"rcb{h}_{i}")
                nc.vector.reciprocal_approx_fast(out=rcb, in_=av_t[i][h][0:CC, :])
                if h == 0:
                    atn_t[i] = atp.tile([128, SW], bf16, tag="atn", name=f"atn_{i}")
                nc.vector.tensor_mul(
                    out=atn_t[i][h * CC:(h + 1) * CC, :],
                    in0=av_t[i][h][CC:2 * CC, :], in1=rcb,
                )

            def emit_proj(i, d, ptag, on_act, split=False, pbufs=None):
                op = ps.tile([128, SW], f32, tag=ptag, bufs=pbufs,
                             name=f"op{d}_{i}")
                if split:
                    # per-head contract halves: the first matmul needs only
                    # head0's normalize, shortening the end-of-kernel chain
                    for h in range(HPC):
                        nc.tensor.matmul(
                            op, lhsT=wct_sb[h * CC:(h + 1) * CC,
                                           d * 128:(d + 1) * 128],
                            rhs=atn_t[i][h * CC:(h + 1) * CC, :],
                            start=(h == 0), stop=(h == 1),
                        )
                else:
                    nc.tensor.matmul(
                        op, lhsT=wct_sb[:, d * 128:(d + 1) * 128],
                        rhs=atn_t[i], start=True, stop=True,
                    )
                ob = osp.tile([128, SW], bf16, tag="ob", name=f"ob{d}_{i}")
                if on_act:
                    nc.scalar.copy(out=ob, in_=op)
                else:
                    nc.vector.tensor_copy(out=ob, in_=op)
                nc.sync.dma_start(
                    out=outp[d * 128:(d + 1) * 128, i * SW:(i + 1) * SW], in_=ob,
                )

            # norm/proj pieces for window i-1, one per early block of window
            # i; ob casts ride on ACT there because DVE is saturated by the
            # normalize chain at window boundaries
            def emit_tail_piece(i_prev, step):
                if step < 2:
                    emit_norm_h(i_prev, step)
                else:
                    emit_proj(i_prev, step - 2, "sc", on_act=True)

            # scores run 2 blocks ahead, AV lags 2 blocks: the exp (plus
            # mask) chain latency (~1.3us) is fully hidden behind ~2.4us of
            # independent PE work
            emit_sc(0)
            emit_sc(1)
            for b in range(NB):
                i, j = blocks[b]
                emit_sc(b + 2)
                emit_exp(b)
                if b > 0:
                    emit_av(b - 1)
                pos = b - win_start[i]
                if i > 0 and pos < 6:
                    emit_tail_piece(i - 1, pos)
            emit_av(NB - 1)
            # final window: norm, then projs over the freed av banks (4
            # independent slots), casts split DVE/ACT to shorten the tail
            emit_norm_h(NW - 1, 0)
            emit_norm_h(NW - 1, 1)
            for d in range(4):
                emit_proj(NW - 1, d, f"av{d % 2}", on_act=(d % 2 == 1),
                          split=True)

    nc.compile()
    return nc


def _get_nc():
    if "nc" not in _CACHE:
        _CACHE["nc"] = _build_bass()
    return _CACHE["nc"]


def _make_in_maps(x, pe, Wqkv, bqkv, Wc):
    bf = ml_dtypes.bfloat16
    tt = np.arange(128)[:, None]   # t (pb partition)
    kk = np.arange(128)[None, :]   # s_local - D (pb free col)
    # keep-mask: pb[t, c] survives iff c >= t (strictly-causal staircase)
    tri = (kk >= tt).astype(np.float32).astype(bf)

    xt_b = {}
    for b in range(B):
        t = (x[:, b, :] + pe[:, b, :]).T.astype(bf)     # [C, S]
        xt_b[b] = np.ascontiguousarray(t.reshape(4, 128, S))

    in_maps = []
    for core in range(NCORE):
        b, hg = core // 4, core % 4
        lo = hg * 128
        W3 = np.concatenate(
            [Wqkv[lo:lo + 128], Wqkv[C + lo:C + lo + 128],
             Wqkv[2 * C + lo:2 * C + lo + 128]])
        w3t = W3.T.reshape(4, 128, 384).transpose(1, 0, 2)
        w3t = np.ascontiguousarray(w3t).astype(bf)
        b3 = np.stack([bqkv[lo:lo + 128], bqkv[C + lo:C + lo + 128]], axis=1)
        b3 = np.ascontiguousarray(b3).astype(np.float32)
        wct = np.ascontiguousarray(Wc[:, lo:lo + 128].T).astype(bf)
        in_maps.append({
            "xt": xt_b[b], "w3t": w3t, "b3": b3,
            "wct": wct, "tri": tri,
        })
    return in_maps


def _numpy_fallback(x, pe, content_mask, Wqkv, bqkv, Wc, bc):
    xpe = (x + pe).astype(np.float32)
    qkv = xpe.reshape(-1, C) @ Wqkv.T + bqkv
    qkv = qkv.reshape(S, B, 3 * C)
    q, k, v = np.split(qkv, 3, axis=-1)
    q = q.reshape(S, B, H, CC)
    k = k.reshape(S, B, H, CC)
    v = v.reshape(S, B, H, CC)
    out = np.empty((S, B, C), np.float32)
    for b in range(B):
        for h in range(H):
            sc = (q[:, b, h] @ k[:, b, h].T) * np.float32(TEMP)
            sc = np.where(content_mask[:, :, b], -np.inf, sc)
            sc = sc - sc.max(axis=1, keepdims=True)
            p = np.exp(sc)
            p /= p.sum(axis=1, keepdims=True)
            out[:, b, h * CC:(h + 1) * CC] = p @ v[:, b, h]
    return (out.reshape(-1, C) @ Wc.T + bc).reshape(S, B, C).astype(np.float32)


def kernel(x, pe, content_mask, pad, Wqkv, bqkv, Wc, bc):
    x = np.asarray(x, dtype=np.float32)
    pe = np.asarray(pe, dtype=np.float32)
    content_mask = np.asarray(content_mask)
    Wqkv = np.asarray(Wqkv, dtype=np.float32)
    bqkv = np.asarray(bqkv, dtype=np.float32)
    Wc = np.asarray(Wc, dtype=np.float32)
    bc = np.asarray(bc, dtype=np.float32)

    idx = np.arange(S)
    causal = idx[None, :] > idx[:, None]
    if not np.array_equal(content_mask, np.broadcast_to(causal[:, :, None], (S, S, B))):
        return _numpy_fallback(x, pe, content_mask, Wqkv, bqkv, Wc, bc)

    from concourse.bass_utils import run_bass_kernel_spmd

    nc = _get_nc()
    in_maps = _make_in_maps(x, pe, Wqkv, bqkv, Wc)
    res = run_bass_kernel_spmd(nc, in_maps, core_ids=list(range(NCORE)))
    out = np.empty((S, B, C), np.float32)
    bc_eff = bc + Wc @ bqkv[2 * C:3 * C]   # v-bias folded through the output proj
    for b in range(B):
        acc = res.results[b * 4]["outp"].astype(np.float32).copy()
        for g in range(1, 4):
            acc += res.results[b * 4 + g]["outp"]
        out[:, b, :] = acc.T + bc_eff
    return out


# revision 44
# speedup vs baseline: 1.0289x; 1.0289x over previous
"""Trainium2 Bass kernel for nn_CompressedCausalAttention.

Sharding: 8 cores = 2 batches x 4 head-groups (2 heads each).
Per-core dataflow (chan-major "T" layouts are (channel partition, seq free)):
  host:    xpe = (x+pe)^T in bf16 (per batch), so the device never sees
           x/pe in f32 and does no adds (DMA 8MB -> 2MB per core).
  phase 1: qT,kT chan-major with bias applied on DVE (tensor_scalar_add,
           keeping ACT free for exps); v seq-major [t, j, h, 128] written
           directly by (xpe^T)-as-lhsT matmuls. Cols 0..63 of each head's
           128-wide v slot are ALL ONES: the AV matmul then emits the
           softmax denominator pre-broadcast 64-wide in rows 0..63, free.
  phase 2: flash-style attention over (i=s-window, j=t-chunk) blocks,
           both heads' scores in one 2-bank PSUM tile, ONE merged exp per
           block on ACT (exp is the throughput co-bottleneck with PE),
           strict-causal staircase applied post-exp as a 0/1 triangle
           multiply on GpSimd (window 0) / DVE (windows 1-3), AV
           accumulation per head with the denominator riding along.
  norm:    1/den straight off AV rows 0..63 via reciprocal_approx_fast
           (DVE, psum in / sbuf out, partition offset 0 - the custom op
           mishandles offset inputs), then one DVE mul -> atn (bf16).
  phase 3: partial output projection outpT = Wc_mine^T-slice @ attnT.
Software pipelining: scores run 2 blocks ahead (PSUM sc-tag rotation
depth 2), AV lags 1 block, and window i's normalize+projection pieces
are spread one-per-block over the first 6 blocks of window i+1; the
final window's projections use the freed AV banks with per-head-split
contractions so they start after head0's normalize alone.
PSUM budget (8 banks): sc 2x2 + av0 2x1 + av1 2x1 = 8.
Host: shards inputs, sums the 4 per-batch partials, adds bc_eff
(v-bias folded through the output projection).
"""

import numpy as np
import ml_dtypes

S, B, C, H = 2048, 2, 512, 8
CC = C // H            # 64
HPC = 2                # heads per core
NCORE = 8
SW = 512               # s window (free dim of score tiles)
TCH = 128              # t chunk (partition dim of score tiles)
NW = S // SW           # 4 windows
TEMP = 1.0 / 8.0       # 1/sqrt(CC)
BIGNEG = -30000.0

_CACHE = {}


def _build_bass():
    import concourse.bass as bass
    import concourse.mybir as mybir
    import concourse.tile as tile
    from concourse import bacc

    f32 = mybir.dt.float32
    bf16 = mybir.dt.bfloat16

    nc = bacc.Bacc("TRN2", target_bir_lowering=False)
    xt = nc.declare_dram_parameter("xt", [4, 128, S], bf16, isOutput=False)
    w3t = nc.declare_dram_parameter("w3t", [128, 4, 384], bf16, isOutput=False)
    b3 = nc.declare_dram_parameter("b3", [128, 2], f32, isOutput=False)
    wct = nc.declare_dram_parameter("wct", [128, C], bf16, isOutput=False)
    tri = nc.declare_dram_parameter("tri", [128, 128], bf16, isOutput=False)
    outp = nc.declare_dram_parameter("outp", [C, S], bf16, isOutput=True)

    Ident = mybir.ActivationFunctionType.Identity
    Exp = mybir.ActivationFunctionType.Exp

    with tile.TileContext(nc) as tc:
        with (
            tc.tile_pool(name="singles", bufs=1) as singles,
            tc.tile_pool(name="pbp", bufs=4) as pbp,
            tc.tile_pool(name="atp", bufs=2) as atp,
            tc.tile_pool(name="rbp", bufs=2) as rbp,
            tc.tile_pool(name="osp", bufs=3) as osp,
            tc.tile_pool(name="ps", bufs=2, space="PSUM") as ps,
        ):
            # ---- inputs: w3t (gates first LDWEIGHTS) leads the Sync
            # queue; window-0 xpe chunks fan out across four engine queues
            # so they land simultaneously; the rest stream on Sync ----
            # w3t split per k-chunk and interleaved with the first-needed
            # xpe chunks so the first q matmul's operands land earliest
            w3t_sb = singles.tile([128, 4, 384], bf16, tag="w3t")
            xpe = singles.tile([128, 4, S], bf16, tag="xpe")
            nc.sync.dma_start(out=w3t_sb[:, 0, :], in_=w3t[:, 0, :])
            nc.sync.dma_start(out=xpe[:, 0, 0:SW], in_=xt[0, :, 0:SW])
            nc.scalar.dma_start(out=xpe[:, 1, 0:SW], in_=xt[1, :, 0:SW])
            nc.gpsimd.dma_start(out=xpe[:, 2, 0:SW], in_=xt[2, :, 0:SW])
            for k in range(1, 4):
                nc.sync.dma_start(out=w3t_sb[:, k, :], in_=w3t[:, k, :])
            nc.sync.dma_start(out=xpe[:, 3, 0:SW], in_=xt[3, :, 0:SW])
            b3_sb = singles.tile([128, 2], f32, tag="b3")
            nc.scalar.dma_start(out=b3_sb, in_=b3[:, :])
            tri_sb = singles.tile([128, 128], bf16, tag="tri")
            nc.gpsimd.dma_start(out=tri_sb, in_=tri[:, :])
            wct_sb = singles.tile([128, C], bf16, tag="wct")
            nc.gpsimd.dma_start(out=wct_sb, in_=wct[:, :])
            for w in range(1, NW):
                sl = slice(w * SW, (w + 1) * SW)
                for k in range(4):
                    nc.sync.dma_start(out=xpe[:, k, sl], in_=xt[k, :, sl])

            qT = singles.tile([128, S], bf16, tag="qT")
            kT = singles.tile([128, S], bf16, tag="kT")
            # v seq-major: [t(128), j(16), h(2), 128]; cols CC..127 of each
            # head slot are all ones, so AV rows CC..127 come out as the
            # softmax denominator already broadcast 64-wide (free on PE).
            vsb = singles.tile([128, 16, HPC, 128], bf16, tag="vsb")
            # big strided memset on GpSimd: keeps DVE free for the window-0
            # q/k bias-adds that gate the qkv pipeline
            nc.gpsimd.memset(vsb[:, :, :, 0:CC], 1.0)

            # p-state pre-warm: dependency-free dummy matmuls start the
            # moment the PE preamble ends (~5.5us), so the clock is fully
            # ramped (3us continuous-busy threshold) and the engine is hot
            # when the first real matmul's DMA lands (~10us). Their garbage
            # PSUM output is overwritten by the first start=True real mm.
            warm = singles.tile([128, SW], bf16, tag="warm")
            nc.vector.memset(warm, 0.0)
            for _ in range(10):
                wp = ps.tile([128, SW], f32, tag="sc", name="wp")
                nc.tensor.matmul(wp, lhsT=warm[:, 0:128], rhs=warm,
                                 start=True, stop=True)
            # ACT warm-up: force the Exp table load (~1.3us) at ~6us on a
            # tiny dummy, instead of lazily inside window 0's first exp
            wexp = singles.tile([1, 8], bf16, tag="wexp")
            nc.scalar.activation(out=wexp, in_=warm[0:1, 0:8], func=Exp,
                                 scale=1.0)

            # ---- phase 1: qkv per window (q/k bias on DVE, ACT is
            # reserved for the attention exps) ----
            for w in range(NW):
                sl = slice(w * SW, (w + 1) * SW)
                qkp = ps.tile([128, 2, SW], f32, tag="sc")
                for blk, dst in ((0, qT), (1, kT)):
                    for k in range(4):
                        nc.tensor.matmul(
                            qkp[:, blk, :],
                            lhsT=w3t_sb[:, k, blk * 128:(blk + 1) * 128],
                            rhs=xpe[:, k, sl],
                            start=(k == 0), stop=(k == 3),
                        )
                    nc.vector.tensor_scalar_add(
                        out=dst[:, sl], in0=qkp[:, blk, :],
                        scalar1=b3_sb[:, blk:blk + 1],
                    )
                vp = ps.tile([128, 4, HPC, CC], f32, tag="sc")
                for tc_ in range(4):
                    t0 = (4 * w + tc_) * TCH
                    for k in range(4):
                        nc.tensor.matmul(
                            vp[:, tc_],
                            lhsT=xpe[:, k, t0:t0 + TCH],
                            rhs=w3t_sb[:, k, 256:384],
                            start=(k == 0), stop=(k == 3),
                        )
                nc.vector.tensor_copy(
                    out=vsb[:, 4 * w:4 * w + 4, :, CC:2 * CC], in_=vp,
                )

            # ---- phase 2+3: attention, flat software-pipelined loop ----
            # per-window block order: j=0 (full, starts the AV accumulation),
            # then the short diagonal blocks (their exp->mask->AV latency is
            # hidden among long neighbors), then long off-diagonal blocks so
            # the window ends with deep PE work in flight
            blocks = []
            win_start = {}
            stop_j = {}
            for i in range(NW):
                js = list(range(4 * i + 4))
                win_start[i] = len(blocks)
                stop_j[i] = js[-1]
                blocks += [(i, j) for j in js]
            NB = len(blocks)
            sc_t = {}
            pb_t = {}
            av_t = {}

            def emit_sc(b):
                if b >= NB or b in sc_t:
                    return
                i, j = blocks[b]
                D = max(0, TCH * j - SW * i)
                t = ps.tile([128, HPC, SW], f32, tag="sc", name=f"sc_{b}")
                for h in range(HPC):
                    nc.tensor.matmul(
                        t[:, h, D:SW],
                        lhsT=kT[h * CC:(h + 1) * CC, j * TCH:(j + 1) * TCH],
                        rhs=qT[h * CC:(h + 1) * CC, i * SW + D:(i + 1) * SW],
                        start=True, stop=True,
                    )
                sc_t[b] = t

            def emit_exp(b):
                i, j = blocks[b]
                D = max(0, TCH * j - SW * i)
                pb = pbp.tile([128, HPC, SW], bf16, tag="pb", name=f"pb_{b}")
                nc.scalar.activation(out=pb[:, :, D:SW], in_=sc_t[b][:, :, D:SW],
                                     func=Exp, scale=TEMP)
                if j >= 4 * i:
                    # strict-causal staircase: zero the masked triangle
                    # post-exp. Window 0 runs it on GpSimd (DVE is busy
                    # with qkv biases/v copies); later windows use DVE,
                    # whose shorter launch latency keeps the lag-1 AV fed
                    # through the short diagonal-block cascade.
                    eng = nc.gpsimd if i == 0 else nc.vector
                    for h in range(HPC):
                        eng.tensor_mul(
                            out=pb[:, h, D:D + TCH], in0=pb[:, h, D:D + TCH],
                            in1=tri_sb,
                        )
                pb_t[b] = pb

            def emit_av(b):
                i, j = blocks[b]
                D = max(0, TCH * j - SW * i)
                if j == 0:
                    av_t[i] = [
                        ps.tile([128, SW], f32, tag=f"av{h}", name=f"av{h}_{i}")
                        for h in range(HPC)
                    ]
                for h in range(HPC):
                    nc.tensor.matmul(
                        av_t[i][h][:, D:SW], lhsT=vsb[:, j, h, :],
                        rhs=pb_t[b][:, h, D:SW],
                        start=(j == 0), stop=(j == stop_j[i]),
                    )
                del pb_t[b]

            # normalize + projection for window i, split into 4 pieces that
            # get interleaved into the next window's block stream
            atn_t = {}

            def emit_norm_h(i, h):
                # v cols 0..63 are all ones, so AV rows 0..63 come out as
                # the softmax denominator already broadcast 64-wide
                rcb = rbp.tile([CC, SW], f32, tag="rcb", name=f"rcb{h}_{i}")
                nc.vector.reciprocal_approx_fast(out=rcb, in_=av_t[i][h][0:CC, :])
                if h == 0:
                    atn_t[i] = atp.tile([128, SW], bf16, tag="atn", name=f"atn_{i}")
                nc.vector.tensor_mul(
                    out=atn_t[i][h * CC:(h + 1) * CC, :],
                    in0=av_t[i][h][CC:2 * CC, :], in1=rcb,
                )

            def emit_proj(i, d, ptag, on_act, split=False, pbufs=None):
                op = ps.tile([128, SW], f32, tag=ptag, bufs=pbufs,
                             name=f"op{d}_{i}")
                if split:
                    # per-head contract halves: the first matmul needs only
                    # head0's normalize, shortening the end-of-kernel chain
                    for h in range(HPC):
                        nc.tensor.matmul(
                            op, lhsT=wct_sb[h * CC:(h + 1) * CC,
                                           d * 128:(d + 1) * 128],
                            rhs=atn_t[i][h * CC:(h + 1) * CC, :],
                            start=(h == 0), stop=(h == 1),
                        )
                else:
                    nc.tensor.matmul(
                        op, lhsT=wct_sb[:, d * 128:(d + 1) * 128],
                        rhs=atn_t[i], start=True, stop=True,
                    )
                ob = osp.tile([128, SW], bf16, tag="ob", name=f"ob{d}_{i}")
                if on_act:
                    nc.scalar.copy(out=ob, in_=op)
                else:
                    nc.vector.tensor_copy(out=ob, in_=op)
                nc.sync.dma_start(
                    out=outp[d * 128:(d + 1) * 128, i * SW:(i + 1) * SW], in_=ob,
                )

            # norm/proj pieces for window i-1, one per early block of window
            # i; ob casts ride on ACT there because DVE is saturated by the
            # normalize chain at window boundaries
            def emit_tail_piece(i_prev, step):
                if step < 2:
                    emit_norm_h(i_prev, step)
                else:
                    emit_proj(i_prev, step - 2, "sc", on_act=True)

            # scores run 2 blocks ahead, AV lags 2 blocks: the exp (plus
            # mask) chain latency (~1.3us) is fully hidden behind ~2.4us of
            # independent PE work
            emit_sc(0)
            emit_sc(1)
            for b in range(NB):
                i, j = blocks[b]
                emit_sc(b + 2)
                emit_exp(b)
                if b > 0:
                    emit_av(b - 1)
                pos = b - win_start[i]
                if i > 0 and pos < 6:
                    emit_tail_piece(i - 1, pos)
            emit_av(NB - 1)
            # final window: norm, then projs over the freed av banks (4
            # independent slots), casts split DVE/ACT to shorten the tail
            emit_norm_h(NW - 1, 0)
            emit_norm_h(NW - 1, 1)
            for d in range(4):
                emit_proj(NW - 1, d, f"av{d % 2}", on_act=(d % 2 == 1),
                          split=True)

    nc.compile()
    return nc


def _get_nc():
    if "nc" not in _CACHE:
        _CACHE["nc"] = _build_bass()
    return _CACHE["nc"]


def _make_in_maps(x, pe, Wqkv, bqkv, Wc):
    bf = ml_dtypes.bfloat16
    tt = np.arange(128)[:, None]   # t (pb partition)
    kk = np.arange(128)[None, :]   # s_local - D (pb free col)
    # keep-mask: pb[t, c] survives iff c >= t (strictly-causal staircase)
    tri = (kk >= tt).astype(np.float32).astype(bf)

    xt_b = {}
    for b in range(B):
        t = (x[:, b, :] + pe[:, b, :]).T.astype(bf)     # [C, S]
        xt_b[b] = np.ascontiguousarray(t.reshape(4, 128, S))

    in_maps = []
    for core in range(NCORE):
        b, hg = core // 4, core % 4
        lo = hg * 128
        W3 = np.concatenate(
            [Wqkv[lo:lo + 128], Wqkv[C + lo:C + lo + 128],
             Wqkv[2 * C + lo:2 * C + lo + 128]])
        w3t = W3.T.reshape(4, 128, 384).transpose(1, 0, 2)
        w3t = np.ascontiguousarray(w3t).astype(bf)
        b3 = np.stack([bqkv[lo:lo + 128], bqkv[C + lo:C + lo + 128]], axis=1)
        b3 = np.ascontiguousarray(b3).astype(np.float32)
        wct = np.ascontiguousarray(Wc[:, lo:lo + 128].T).astype(bf)
        in_maps.append({
            "xt": xt_b[b], "w3t": w3t, "b3": b3,
            "wct": wct, "tri": tri,
        })
    return in_maps


def _numpy_fallback(x, pe, content_mask, Wqkv, bqkv, Wc, bc):
    xpe = (x + pe).astype(np.float32)
    qkv = xpe.reshape(-1, C) @ Wqkv.T + bqkv
    qkv = qkv.reshape(S, B, 3 * C)
    q, k, v = np.split(qkv, 3, axis=-1)
    q = q.reshape(S, B, H, CC)
    k = k.reshape(S, B, H, CC)
    v = v.reshape(S, B, H, CC)
    out = np.empty((S, B, C), np.float32)
    for b in range(B):
        for h in range(H):
            sc = (q[:, b, h] @ k[:, b, h].T) * np.float32(TEMP)
            sc = np.where(content_mask[:, :, b], -np.inf, sc)
            sc = sc - sc.max(axis=1, keepdims=True)
            p = np.exp(sc)
            p /= p.sum(axis=1, keepdims=True)
            out[:, b, h * CC:(h + 1) * CC] = p @ v[:, b, h]
    return (out.reshape(-1, C) @ Wc.T + bc).reshape(S, B, C).astype(np.float32)


def kernel(x, pe, content_mask, pad, Wqkv, bqkv, Wc, bc):
    x = np.asarray(x, dtype=np.float32)
    pe = np.asarray(pe, dtype=np.float32)
    content_mask = np.asarray(content_mask)
    Wqkv = np.asarray(Wqkv, dtype=np.float32)
    bqkv = np.asarray(bqkv, dtype=np.float32)
    Wc = np.asarray(Wc, dtype=np.float32)
    bc = np.asarray(bc, dtype=np.float32)

    idx = np.arange(S)
    causal = idx[None, :] > idx[:, None]
    if not np.array_equal(content_mask, np.broadcast_to(causal[:, :, None], (S, S, B))):
        return _numpy_fallback(x, pe, content_mask, Wqkv, bqkv, Wc, bc)

    from concourse.bass_utils import run_bass_kernel_spmd

    nc = _get_nc()
    in_maps = _make_in_maps(x, pe, Wqkv, bqkv, Wc)
    res = run_bass_kernel_spmd(nc, in_maps, core_ids=list(range(NCORE)))
    out = np.empty((S, B, C), np.float32)
    bc_eff = bc + Wc @ bqkv[2 * C:3 * C]   # v-bias folded through the output proj
    for b in range(B):
        acc = res.results[b * 4]["outp"].astype(np.float32).copy()
        for g in range(1, 4):
            acc += res.results[b * 4 + g]["outp"]
        out[:, b, :] = acc.T + bc_eff
    return out
